# revision 18
# baseline (speedup 1.0000x reference)
"""Trainium2 Bass kernel for nn_NeuralODE_15556371546632.

RK4 integration of x' = MLP(x) (2 -> 128 -> 128 -> 2, relu) for M=4096
trajectories, N=200 timesteps.  Data-parallel over 8 NeuronCores
(512 trajectories/core); each core splits its batch into CHUNKS
independent column-chunks so the Tile scheduler can overlap engines
across the serial dependency chain of one chunk.

Math (per step n, step size h = t[n+1]-t[n], batch stored column-major
xT [2, B]):
    k_i = W3.T h2_i + b3,  h2_i = relu(W2.T h1_i + b2),
    h1_i = relu(pre_i + bias_i)
  with the RK4 stage updates fused into PSUM accumulation:
    pre_1 = W1.T x
    pre_2 = W1.T x + (h/2 * W3W1).T h2_1      (bias_2 = b1 + h/2 * W1.T b3)
    pre_3 = W1.T x + (h/2 * W3W1).T h2_2
    pre_4 = W1.T x + (h   * W3W1).T h2_3      (bias_4 = b1 + h * W1.T b3)
    S     = (h/6*W3).T h2_1 + (h/3*W3).T h2_2 + (h/3*W3).T h2_3 + (h/6*W3).T h2_4
    x'    = x + S + h*b3
All matmuls run with float16 operands (fp32 accumulate in PSUM):
validated end-to-end rel err ~6e-4 vs fp32 reference (numpy sim).
float32r was measured on HW to run at fp32 speed (~4 cyc/col,
fp32_mode=HIGH); fp16 streams 1 col/cycle and gets fast weight load.
The x state itself propagates in full fp32; a rounded fp16 copy feeds
the matmuls.

PSUM budget (8 banks, 2 chunks): per chunk 2 pre + 1 E + 1 S slots.
The pre-bank mmU (W1.T x) restarts are emitted just-in-time (one eval
ahead of their accumulate) so only 2 pre banks per chunk are ever live.
"""

import numpy as np

M = 4096
N_STEPS = 199  # N-1
H = 128
N_CORES = 8
B_CORE = M // N_CORES          # 512 trajectories per core
CHUNKS = 2
B_CHUNK = B_CORE // CHUNKS     # 256 columns per chunk

_compiled = None


def _enable_ldw_opt():
    import os
    if not os.environ.get("BASS_LDW_OPT"):
        return
    import concourse.bass_utils as bu
    if getattr(bu, "_ldw_opt_patched", False):
        return
    orig = bu.run_command
    def patched(argv, **kw):
        argv = ["--enable-ldw-opt=true" if a == "--enable-ldw-opt=false" else a
                for a in argv]
        return orig(argv, **kw)
    bu.run_command = patched
    bu._ldw_opt_patched = True


def _build_program():
    from contextlib import ExitStack

    import concourse.bacc as bacc
    import concourse.tile as tile
    from concourse import mybir

    f32 = mybir.dt.float32
    f16 = mybir.dt.float16
    Alu = mybir.AluOpType
    Act = mybir.ActivationFunctionType

    _enable_ldw_opt()
    nc = bacc.Bacc(
        "TRN2",
        target_bir_lowering=False,
        debug=False,
        enable_asserts=True,
        num_devices=N_CORES,
    )

    # ---- DRAM I/O ----
    x0T_d = nc.dram_tensor("x0T", [2, B_CORE], f32, kind="ExternalInput").ap()
    w1_d = nc.dram_tensor("w1", [2, H], f16, kind="ExternalInput").ap()
    w2_d = nc.dram_tensor("w2", [H, H], f16, kind="ExternalInput").ap()
    # per-step scaled (W3@W1): [n] -> (h/2)*Wf ; and h*Wf
    wfa_d = nc.dram_tensor("wfa", [N_STEPS, H, H], f16, kind="ExternalInput").ap()
    wfb_d = nc.dram_tensor("wfb", [N_STEPS, H, H], f16, kind="ExternalInput").ap()
    # per-step scaled W3 columns, interleaved [128, N_STEPS*4]:
    # cols 4n:4n+2 = (h/6)W3, 4n+2:4n+4 = (h/3)W3
    w3s_d = nc.dram_tensor("w3s", [H, N_STEPS * 4], f16, kind="ExternalInput").ap()
    # biases: [128, N_STEPS] columns; biasB = b1 + (h/2) W1.T b3, biasD = b1 + h W1.T b3
    biasA_d = nc.dram_tensor("biasA", [H, 1], f32, kind="ExternalInput").ap()
    biasB_d = nc.dram_tensor("biasB", [H, N_STEPS], f32, kind="ExternalInput").ap()
    biasD_d = nc.dram_tensor("biasD", [H, N_STEPS], f32, kind="ExternalInput").ap()
    b2_d = nc.dram_tensor("b2", [H, 1], f32, kind="ExternalInput").ap()
    hb3_d = nc.dram_tensor("hb3", [2, N_STEPS], f32, kind="ExternalInput").ap()
    # output: steps 1..199, feature-major [n, 2, B_CORE]
    y_d = nc.dram_tensor("y", [N_STEPS, 2, B_CORE], f32, kind="ExternalOutput").ap()

    with tile.TileContext(nc) as tc, ExitStack() as ctx:
        consts = ctx.enter_context(tc.tile_pool(name="consts", bufs=1))
        wf_pool = ctx.enter_context(tc.tile_pool(name="wf", bufs=3))
        act_pool = ctx.enter_context(tc.tile_pool(name="acts", bufs=4))
        x_pool = ctx.enter_context(tc.tile_pool(name="xs", bufs=4))
        psum = ctx.enter_context(tc.tile_pool(name="psum", bufs=1, space="PSUM"))

        # ---- load constants ----
        w1_s = consts.tile([2, H], f16)
        nc.sync.dma_start(w1_s[:], w1_d[:])
        w2_s = consts.tile([H, H], f16)
        nc.sync.dma_start(w2_s[:], w2_d[:])
        w3s_s = consts.tile([H, N_STEPS * 4], f16)
        nc.sync.dma_start(w3s_s[:], w3s_d[:])
        biasA_s = consts.tile([H, 1], f32)
        nc.sync.dma_start(biasA_s[:], biasA_d[:])
        biasB_s = consts.tile([H, N_STEPS], f32)
        nc.sync.dma_start(biasB_s[:], biasB_d[:])
        biasD_s = consts.tile([H, N_STEPS], f32)
        nc.sync.dma_start(biasD_s[:], biasD_d[:])
        b2_s = consts.tile([H, 1], f32)
        nc.sync.dma_start(b2_s[:], b2_d[:])
        hb3_s = consts.tile([2, N_STEPS], f32)
        nc.sync.dma_start(hb3_s[:], hb3_d[:])

        # initial x chunks: full-precision state + rounded fp16 copy
        xc, xrc = [], []
        for c in range(CHUNKS):
            xt = x_pool.tile([2, B_CHUNK], f32, name=f"x_c{c}", tag=f"x{c}")
            nc.sync.dma_start(xt[:], x0T_d[:, c * B_CHUNK : (c + 1) * B_CHUNK])
            xr = x_pool.tile([2, B_CHUNK], f16, name=f"xr_c{c}", tag=f"xr{c}")
            nc.vector.tensor_copy(xr[:], xt[:])
            xc.append(xt)
            xrc.append(xr)

        def mm(out, lhsT, rhs, start, stop):
            nc.tensor.matmul(out, lhsT, rhs, start=start, stop=stop)

        # per-chunk step state machines, advanced stage-by-stage so that
        # same-weight matmuls of the two chunks sit adjacent in the PE
        # queue (LDW sharing, PE density) and the activation engines
        # ping-pong between the chunk chains.
        class ChunkStep:
            def __init__(self, c, n, wfa, wfb):
                self.c, self.n = c, n
                self.wfa, self.wfb = wfa, wfb
                self.pre = [None] * 5  # pre banks 1..4
                self.S = None
                self.h1 = None
                self.h2 = None

            def t(self, pool_tag, shape, dtype, nm):
                bufs = {"pre": 2, "e": 1, "s": 1}[pool_tag]
                return psum.tile(
                    shape, dtype, name=f"{nm}_{self.n}_{self.c}",
                    tag=f"{pool_tag}{self.c}", bufs=bufs,
                )

            def seed12(self):
                # pre1 and pre2 share one PSUM accumulation group: seed
                # W1.T x (start, no stop); h1_1 reads the seed value
                # mid-group, then the i=1 wf-accumulate lands on top to
                # form pre2 in the same bank.
                self.pre[1] = self.t("pre", [H, B_CHUNK], f32, "U")
                mm(self.pre[1][:], w1_s[:], xrc[self.c][:],
                   start=True, stop=False)
                self.pre[2] = self.pre[1]
                self.S = self.t("s", [2, B_CHUNK], f32, "S")

            def seed(self, j):
                self.pre[j] = self.t("pre", [H, B_CHUNK], f32, f"P{j}")
                mm(self.pre[j][:], w1_s[:], xrc[self.c][:],
                   start=True, stop=False)

            def act_h1(self, i):
                # split each relu into two half-width ops on ACT and DVE in
                # parallel: chain-link latency drops from the full-FD act
                # cost to the half-FD cost (~360ns vs ~474ns)
                c, n = self.c, self.n
                bB = biasB_s[:, n : n + 1]
                bD = biasD_s[:, n : n + 1]
                bias = {1: biasA_s[:, 0:1], 2: bB, 3: bB, 4: bD}[i]
                h1 = act_pool.tile([H, B_CHUNK], f16, name=f"h1_{n}_{c}{i}",
                                   tag=f"h1{c}")
                HB = B_CHUNK // 2
                lo = slice(0, HB)
                hi = slice(HB, B_CHUNK)
                a, b = (lo, hi) if c == 0 else (hi, lo)
                nc.scalar.activation(h1[:, a], self.pre[i][:, a], Act.Relu,
                                     bias=bias)
                nc.vector.tensor_scalar(h1[:, b], self.pre[i][:, b], bias,
                                        0.0, Alu.add, Alu.max)
                self.h1 = h1

            def mm_E(self, i):
                E = self.t("e", [H, B_CHUNK], f32, f"E{i}")
                mm(E[:], w2_s[:], self.h1[:], start=True, stop=True)
                self.E = E

            def act_h2(self, i):
                c, n = self.c, self.n
                h2 = act_pool.tile([H, B_CHUNK], f16, name=f"h2_{n}_{c}{i}",
                                   tag=f"h2{c}")
                HB = B_CHUNK // 2
                lo = slice(0, HB)
                hi = slice(HB, B_CHUNK)
                a, b = (hi, lo) if c == 0 else (lo, hi)
                nc.scalar.activation(h2[:, a], self.E[:, a], Act.Relu,
                                     bias=b2_s[:, 0:1])
                nc.vector.tensor_scalar(h2[:, b], self.E[:, b], b2_s[:, 0:1],
                                        0.0, Alu.add, Alu.max)
                self.h2 = h2

            def mm_wf(self, i):
                if i < 4:
                    wf = self.wfa if i < 3 else self.wfb
                    mm(self.pre[i + 1][:], wf[:], self.h2[:],
                       start=False, stop=True)

            def mm_S(self, i):
                n = self.n
                w3col = w3s_s[:, 4 * n : 4 * n + 2] if i in (1, 4) \
                    else w3s_s[:, 4 * n + 2 : 4 * n + 4]
                mm(self.S[:], w3col[:], self.h2[:],
                   start=(i == 1), stop=(i == 4))

            def finish(self):
                # fp16 copy first: the next step's seed matmuls depend only
                # on xnr, so it leads; the fp32 state update follows off the
                # critical path.
                c, n = self.c, self.n
                hb3c = hb3_s[:, n : n + 1]
                xnr = x_pool.tile([2, B_CHUNK], f16, name=f"xr_{n}_{c}",
                                  tag=f"xr{c}")
                nc.vector.scalar_tensor_tensor(
                    xnr[:], self.S[:], hb3c, xc[c][:], Alu.add, Alu.add
                )
                xn = x_pool.tile([2, B_CHUNK], f32, name=f"x_{n}_{c}", tag=f"x{c}")
                nc.vector.scalar_tensor_tensor(
                    xn[:], self.S[:], hb3c, xc[c][:], Alu.add, Alu.add
                )
                nc.sync.dma_start(
                    y_d[n, :, c * B_CHUNK : (c + 1) * B_CHUNK], xn[:]
                )
                xc[c] = xn
                xrc[c] = xnr

        for n in range(N_STEPS):
            wfa = wf_pool.tile([H, H], f16, name=f"wfa_{n}", tag="wfa")
            nc.sync.dma_start(wfa[:], wfa_d[n])
            wfb = wf_pool.tile([H, H], f16, name=f"wfb_{n}", tag="wfb")
            nc.sync.dma_start(wfb[:], wfb_d[n])
            steps = [ChunkStep(c, n, wfa, wfb) for c in range(CHUNKS)]
            for s in steps:
                s.seed12()
            for i in (1, 2, 3, 4):
                for s in steps:
                    s.act_h1(i)
                for s in steps:
                    s.mm_E(i)
                for s in steps:
                    s.act_h2(i)
                if i < 3:
                    # seed pre bank for eval i+2 (W1.T x restart) before the
                    # wf accumulate of eval i+1 can land on it
                    for s in steps:
                        s.seed(i + 2)
                for s in steps:
                    s.mm_wf(i)
                for s in steps:
                    s.mm_S(i)
            for s in steps:
                s.finish()

    nc.compile()
    return nc


def _prep_inputs(x0, t, W1, b1, W2, b2, W3, b3):
    """Host-side derived tensors (weights fp16, biases fp32)."""
    f32 = np.float32
    f16 = np.float16
    hs = (t[1:] - t[:-1]).astype(f32)  # [199], same op order as reference
    Wf = (W3.astype(np.float64) @ W1.astype(np.float64))  # [128,128]
    wfa = np.empty((N_STEPS, H, H), f16)
    wfb = np.empty((N_STEPS, H, H), f16)
    w3s = np.empty((H, N_STEPS * 4), f16)
    biasB = np.empty((H, N_STEPS), f32)
    biasD = np.empty((H, N_STEPS), f32)
    hb3 = np.empty((2, N_STEPS), f32)
    w1b3 = (W1.astype(np.float64).T @ b3.astype(np.float64))  # [128]
    b1_64 = b1.astype(np.float64)
    W3_64 = W3.astype(np.float64)
    for n in range(N_STEPS):
        h = float(hs[n])
        wfa[n] = ((h / 2.0) * Wf).astype(f16)
        wfb[n] = (h * Wf).astype(f16)
        w3s[:, 4 * n : 4 * n + 2] = ((h / 6.0) * W3_64).astype(f16)
        w3s[:, 4 * n + 2 : 4 * n + 4] = ((h / 3.0) * W3_64).astype(f16)
        biasB[:, n] = (b1_64 + (h / 2.0) * w1b3).astype(f32)
        biasD[:, n] = (b1_64 + h * w1b3).astype(f32)
        hb3[:, n] = (h * b3.astype(np.float64)).astype(f32)
    shared = {
        "w1": np.ascontiguousarray(W1.astype(f16)),
        "w2": np.ascontiguousarray(W2.astype(f16)),
        "wfa": wfa,
        "wfb": wfb,
        "w3s": w3s,
        "biasA": np.ascontiguousarray(b1.astype(f32).reshape(H, 1)),
        "biasB": biasB,
        "biasD": biasD,
        "b2": np.ascontiguousarray(b2.astype(f32).reshape(H, 1)),
        "hb3": hb3,
    }
    in_maps = []
    for c in range(N_CORES):
        m = dict(shared)
        m["x0T"] = np.ascontiguousarray(
            x0[c * B_CORE : (c + 1) * B_CORE].astype(f32).T
        )
        in_maps.append(m)
    return in_maps


def kernel(x0, t, W1, b1, W2, b2, W3, b3):
    global _compiled
    from concourse.bass_utils import run_bass_kernel_spmd

    if _compiled is None:
        _compiled = _build_program()
    nc = _compiled

    in_maps = _prep_inputs(x0, t, W1, b1, W2, b2, W3, b3)
    res = run_bass_kernel_spmd(nc, in_maps, list(range(N_CORES))).results

    out = np.empty((N_STEPS + 1, M, 2), np.float32)
    out[0] = x0
    for c in range(N_CORES):
        y = res[c]["y"]  # [199, 2, 512]
        out[1:, c * B_CORE : (c + 1) * B_CORE, :] = y.transpose(0, 2, 1)
    return out



# revision 19
# speedup vs baseline: 1.1941x; 1.1941x over previous
"""Trainium2 Bass kernel for nn_NeuralODE_15556371546632.

RK4 integration of x' = MLP(x) (2 -> 128 -> 128 -> 2, relu) for M=4096
trajectories, N=200 timesteps.  Data-parallel over 8 NeuronCores
(512 trajectories/core); each core splits its batch into CHUNKS
independent column-chunks so the Tile scheduler can overlap engines
across the serial dependency chain of one chunk.

Math (per step n, step size h = t[n+1]-t[n], batch stored column-major
xT [2, B]):
    k_i = W3.T h2_i + b3,  h2_i = relu(W2.T h1_i + b2),
    h1_i = relu(pre_i + bias_i)
  with the RK4 stage updates fused into PSUM accumulation:
    pre_1 = W1.T x
    pre_2 = W1.T x + (h/2 * W3W1).T h2_1      (bias_2 = b1 + h/2 * W1.T b3)
    pre_3 = W1.T x + (h/2 * W3W1).T h2_2
    pre_4 = W1.T x + (h   * W3W1).T h2_3      (bias_4 = b1 + h * W1.T b3)
    S     = (h/6*W3).T h2_1 + (h/3*W3).T h2_2 + (h/3*W3).T h2_3 + (h/6*W3).T h2_4
    x'    = x + S + h*b3
All matmuls run with float16 operands (fp32 accumulate in PSUM):
validated end-to-end rel err ~6e-4 vs fp32 reference (numpy sim).
float32r was measured on HW to run at fp32 speed (~4 cyc/col,
fp32_mode=HIGH); fp16 streams 1 col/cycle and gets fast weight load.
The x state itself propagates in full fp32; a rounded fp16 copy feeds
the matmuls.

PSUM budget (8 banks, 2 chunks): per chunk 2 pre + 1 E + 1 S slots.
The pre-bank mmU (W1.T x) restarts are emitted just-in-time (one eval
ahead of their accumulate) so only 2 pre banks per chunk are ever live.
"""

import numpy as np

M = 4096
N_STEPS = 199  # N-1
H = 128
N_CORES = 8
B_CORE = M // N_CORES          # 512 trajectories per core
CHUNKS = 2
B_CHUNK = B_CORE // CHUNKS     # 256 columns per chunk

_compiled = None


def _enable_ldw_opt():
    import os
    if not os.environ.get("BASS_LDW_OPT"):
        return
    import concourse.bass_utils as bu
    if getattr(bu, "_ldw_opt_patched", False):
        return
    orig = bu.run_command
    def patched(argv, **kw):
        argv = ["--enable-ldw-opt=true" if a == "--enable-ldw-opt=false" else a
                for a in argv]
        return orig(argv, **kw)
    bu.run_command = patched
    bu._ldw_opt_patched = True


def _build_program():
    from contextlib import ExitStack

    import concourse.bacc as bacc
    import concourse.tile as tile
    from concourse import mybir

    f32 = mybir.dt.float32
    f16 = mybir.dt.float16
    Alu = mybir.AluOpType
    Act = mybir.ActivationFunctionType

    _enable_ldw_opt()
    nc = bacc.Bacc(
        "TRN2",
        target_bir_lowering=False,
        debug=False,
        enable_asserts=True,
        num_devices=N_CORES,
    )

    # ---- DRAM I/O ----
    x0T_d = nc.dram_tensor("x0T", [2, B_CORE], f32, kind="ExternalInput").ap()
    w1_d = nc.dram_tensor("w1", [2, H], f16, kind="ExternalInput").ap()
    w2_d = nc.dram_tensor("w2", [H, H], f16, kind="ExternalInput").ap()
    # per-step scaled (W3@W1): [n] -> (h/2)*Wf ; and h*Wf
    wfa_d = nc.dram_tensor("wfa", [N_STEPS, H, H], f16, kind="ExternalInput").ap()
    wfb_d = nc.dram_tensor("wfb", [N_STEPS, H, H], f16, kind="ExternalInput").ap()
    # per-step scaled W3 columns, interleaved [128, N_STEPS*4]:
    # cols 4n:4n+2 = (h/6)W3, 4n+2:4n+4 = (h/3)W3
    w3s_d = nc.dram_tensor("w3s", [H, N_STEPS * 4], f16, kind="ExternalInput").ap()
    # biases: [128, N_STEPS] columns; biasB = b1 + (h/2) W1.T b3, biasD = b1 + h W1.T b3
    biasA_d = nc.dram_tensor("biasA", [H, 1], f32, kind="ExternalInput").ap()
    biasB_d = nc.dram_tensor("biasB", [H, N_STEPS], f32, kind="ExternalInput").ap()
    biasD_d = nc.dram_tensor("biasD", [H, N_STEPS], f32, kind="ExternalInput").ap()
    b2_d = nc.dram_tensor("b2", [H, 1], f32, kind="ExternalInput").ap()
    hb3_d = nc.dram_tensor("hb3", [2, N_STEPS], f32, kind="ExternalInput").ap()
    # output: steps 1..199, feature-major [n, 2, B_CORE]
    y_d = nc.dram_tensor("y", [N_STEPS, 2, B_CORE], f32, kind="ExternalOutput").ap()

    with tile.TileContext(nc) as tc, ExitStack() as ctx:
        consts = ctx.enter_context(tc.tile_pool(name="consts", bufs=1))
        wf_pool = ctx.enter_context(tc.tile_pool(name="wf", bufs=3))
        act_pool = ctx.enter_context(tc.tile_pool(name="acts", bufs=4))
        x_pool = ctx.enter_context(tc.tile_pool(name="xs", bufs=4))
        psum = ctx.enter_context(tc.tile_pool(name="psum", bufs=1, space="PSUM"))

        # ---- load constants ----
        w1_s = consts.tile([2, H], f16)
        nc.sync.dma_start(w1_s[:], w1_d[:])
        w2_s = consts.tile([H, H], f16)
        nc.sync.dma_start(w2_s[:], w2_d[:])
        w3s_s = consts.tile([H, N_STEPS * 4], f16)
        nc.sync.dma_start(w3s_s[:], w3s_d[:])
        biasA_s = consts.tile([H, 1], f32)
        nc.sync.dma_start(biasA_s[:], biasA_d[:])
        biasB_s = consts.tile([H, N_STEPS], f32)
        nc.sync.dma_start(biasB_s[:], biasB_d[:])
        biasD_s = consts.tile([H, N_STEPS], f32)
        nc.sync.dma_start(biasD_s[:], biasD_d[:])
        b2_s = consts.tile([H, 1], f32)
        nc.sync.dma_start(b2_s[:], b2_d[:])
        hb3_s = consts.tile([2, N_STEPS], f32)
        nc.sync.dma_start(hb3_s[:], hb3_d[:])

        # initial x chunks: full-precision state + rounded fp16 copy
        xc, xrc = [], []
        for c in range(CHUNKS):
            xt = x_pool.tile([2, B_CHUNK], f32, name=f"x_c{c}", tag=f"x{c}")
            nc.sync.dma_start(xt[:], x0T_d[:, c * B_CHUNK : (c + 1) * B_CHUNK])
            xr = x_pool.tile([2, B_CHUNK], f16, name=f"xr_c{c}", tag=f"xr{c}")
            nc.vector.tensor_copy(xr[:], xt[:])
            xc.append(xt)
            xrc.append(xr)

        def mm(out, lhsT, rhs, start, stop):
            nc.tensor.matmul(out, lhsT, rhs, start=start, stop=stop)

        # per-chunk step state machines, advanced stage-by-stage so that
        # same-weight matmuls of the two chunks sit adjacent in the PE
        # queue (LDW sharing, PE density) and the activation engines
        # ping-pong between the chunk chains.
        class ChunkStep:
            def __init__(self, c, n, wfa, wfb):
                self.c, self.n = c, n
                self.wfa, self.wfb = wfa, wfb
                self.pre = [None] * 5  # pre banks 1..4
                self.S = None
                self.h1 = None
                self.h2 = None

            def t(self, pool_tag, shape, dtype, nm):
                bufs = {"pre": 2, "e": 1, "s": 1}[pool_tag]
                return psum.tile(
                    shape, dtype, name=f"{nm}_{self.n}_{self.c}",
                    tag=f"{pool_tag}{self.c}", bufs=bufs,
                )

            def seed12(self):
                # pre1 and pre2 share one PSUM accumulation group: seed
                # W1.T x (start, no stop); h1_1 reads the seed value
                # mid-group, then the i=1 wf-accumulate lands on top to
                # form pre2 in the same bank.
                self.pre[1] = self.t("pre", [H, B_CHUNK], f32, "U")
                mm(self.pre[1][:], w1_s[:], xrc[self.c][:],
                   start=True, stop=False)
                self.pre[2] = self.pre[1]
                self.S = self.t("s", [2, B_CHUNK], f32, "S")

            def seed(self, j):
                self.pre[j] = self.t("pre", [H, B_CHUNK], f32, f"P{j}")
                mm(self.pre[j][:], w1_s[:], xrc[self.c][:],
                   start=True, stop=False)

            def act_h1(self, i):
                c, n = self.c, self.n
                bB = biasB_s[:, n : n + 1]
                bD = biasD_s[:, n : n + 1]
                bias = {1: biasA_s[:, 0:1], 2: bB, 3: bB, 4: bD}[i]
                h1 = act_pool.tile([H, B_CHUNK], f16, name=f"h1_{n}_{c}{i}",
                                   tag=f"h1{c}")
                # engine split by chunk parity so the two chunk chains never
                # queue behind each other on the same activation engine
                if c == 0:
                    nc.scalar.activation(h1[:], self.pre[i][:], Act.Relu,
                                         bias=bias)
                else:
                    nc.vector.tensor_scalar(h1[:], self.pre[i][:], bias, 0.0,
                                            Alu.add, Alu.max)
                self.h1 = h1

            def mm_E(self, i):
                E = self.t("e", [H, B_CHUNK], f32, f"E{i}")
                mm(E[:], w2_s[:], self.h1[:], start=True, stop=True)
                self.E = E

            def act_h2(self, i):
                c, n = self.c, self.n
                h2 = act_pool.tile([H, B_CHUNK], f16, name=f"h2_{n}_{c}{i}",
                                   tag=f"h2{c}")
                if c == 0:
                    nc.vector.tensor_scalar(h2[:], self.E[:], b2_s[:, 0:1],
                                            0.0, Alu.add, Alu.max)
                else:
                    nc.scalar.activation(h2[:], self.E[:], Act.Relu,
                                         bias=b2_s[:, 0:1])
                self.h2 = h2

            def mm_wf(self, i):
                if i < 4:
                    wf = self.wfa if i < 3 else self.wfb
                    mm(self.pre[i + 1][:], wf[:], self.h2[:],
                       start=False, stop=True)

            def mm_S(self, i):
                n = self.n
                w3col = w3s_s[:, 4 * n : 4 * n + 2] if i in (1, 4) \
                    else w3s_s[:, 4 * n + 2 : 4 * n + 4]
                mm(self.S[:], w3col[:], self.h2[:],
                   start=(i == 1), stop=(i == 4))

            def finish(self):
                # fp16 copy first: the next step's seed matmuls depend only
                # on xnr, so it leads; the fp32 state update follows off the
                # critical path.
                c, n = self.c, self.n
                hb3c = hb3_s[:, n : n + 1]
                xnr = x_pool.tile([2, B_CHUNK], f16, name=f"xr_{n}_{c}",
                                  tag=f"xr{c}")
                nc.vector.scalar_tensor_tensor(
                    xnr[:], self.S[:], hb3c, xc[c][:], Alu.add, Alu.add
                )
                xn = x_pool.tile([2, B_CHUNK], f32, name=f"x_{n}_{c}", tag=f"x{c}")
                nc.vector.scalar_tensor_tensor(
                    xn[:], self.S[:], hb3c, xc[c][:], Alu.add, Alu.add
                )
                nc.sync.dma_start(
                    y_d[n, :, c * B_CHUNK : (c + 1) * B_CHUNK], xn[:]
                )
                xc[c] = xn
                xrc[c] = xnr

        for n in range(N_STEPS):
            wfa = wf_pool.tile([H, H], f16, name=f"wfa_{n}", tag="wfa")
            nc.sync.dma_start(wfa[:], wfa_d[n])
            wfb = wf_pool.tile([H, H], f16, name=f"wfb_{n}", tag="wfb")
            nc.sync.dma_start(wfb[:], wfb_d[n])
            steps = [ChunkStep(c, n, wfa, wfb) for c in range(CHUNKS)]
            for s in steps:
                s.seed12()
            for i in (1, 2, 3, 4):
                for s in steps:
                    s.act_h1(i)
                for s in steps:
                    s.mm_E(i)
                for s in steps:
                    s.act_h2(i)
                if i < 3:
                    # seed pre bank for eval i+2 (W1.T x restart) before the
                    # wf accumulate of eval i+1 can land on it
                    for s in steps:
                        s.seed(i + 2)
                for s in steps:
                    s.mm_wf(i)
                for s in steps:
                    s.mm_S(i)
            for s in steps:
                s.finish()

    nc.compile()
    return nc


def _prep_inputs(x0, t, W1, b1, W2, b2, W3, b3):
    """Host-side derived tensors (weights fp16, biases fp32)."""
    f32 = np.float32
    f16 = np.float16
    hs = (t[1:] - t[:-1]).astype(f32)  # [199], same op order as reference
    Wf = (W3.astype(np.float64) @ W1.astype(np.float64))  # [128,128]
    wfa = np.empty((N_STEPS, H, H), f16)
    wfb = np.empty((N_STEPS, H, H), f16)
    w3s = np.empty((H, N_STEPS * 4), f16)
    biasB = np.empty((H, N_STEPS), f32)
    biasD = np.empty((H, N_STEPS), f32)
    hb3 = np.empty((2, N_STEPS), f32)
    w1b3 = (W1.astype(np.float64).T @ b3.astype(np.float64))  # [128]
    b1_64 = b1.astype(np.float64)
    W3_64 = W3.astype(np.float64)
    for n in range(N_STEPS):
        h = float(hs[n])
        wfa[n] = ((h / 2.0) * Wf).astype(f16)
        wfb[n] = (h * Wf).astype(f16)
        w3s[:, 4 * n : 4 * n + 2] = ((h / 6.0) * W3_64).astype(f16)
        w3s[:, 4 * n + 2 : 4 * n + 4] = ((h / 3.0) * W3_64).astype(f16)
        biasB[:, n] = (b1_64 + (h / 2.0) * w1b3).astype(f32)
        biasD[:, n] = (b1_64 + h * w1b3).astype(f32)
        hb3[:, n] = (h * b3.astype(np.float64)).astype(f32)
    shared = {
        "w1": np.ascontiguousarray(W1.astype(f16)),
        "w2": np.ascontiguousarray(W2.astype(f16)),
        "wfa": wfa,
        "wfb": wfb,
        "w3s": w3s,
        "biasA": np.ascontiguousarray(b1.astype(f32).reshape(H, 1)),
        "biasB": biasB,
        "biasD": biasD,
        "b2": np.ascontiguousarray(b2.astype(f32).reshape(H, 1)),
        "hb3": hb3,
    }
    in_maps = []
    for c in range(N_CORES):
        m = dict(shared)
        m["x0T"] = np.ascontiguousarray(
            x0[c * B_CORE : (c + 1) * B_CORE].astype(f32).T
        )
        in_maps.append(m)
    return in_maps


def kernel(x0, t, W1, b1, W2, b2, W3, b3):
    global _compiled
    from concourse.bass_utils import run_bass_kernel_spmd

    if _compiled is None:
        _compiled = _build_program()
    nc = _compiled

    in_maps = _prep_inputs(x0, t, W1, b1, W2, b2, W3, b3)
    res = run_bass_kernel_spmd(nc, in_maps, list(range(N_CORES))).results

    out = np.empty((N_STEPS + 1, M, 2), np.float32)
    out[0] = x0
    for c in range(N_CORES):
        y = res[c]["y"]  # [199, 2, 512]
        out[1:, c * B_CORE : (c + 1) * B_CORE, :] = y.transpose(0, 2, 1)
    return out



# revision 20
# speedup vs baseline: 1.2356x; 1.0348x over previous
"""Trainium2 Bass kernel for nn_NeuralODE_15556371546632.

RK4 integration of x' = MLP(x) (2 -> 128 -> 128 -> 2, relu) for M=4096
trajectories, N=200 timesteps.  Data-parallel over 8 NeuronCores
(512 trajectories/core); each core splits its batch into CHUNKS
independent column-chunks so the Tile scheduler can overlap engines
across the serial dependency chain of one chunk.

Math (per step n, step size h = t[n+1]-t[n], batch stored column-major
xT [2, B]):
    k_i = W3.T h2_i + b3,  h2_i = relu(W2.T h1_i + b2),
    h1_i = relu(pre_i + bias_i)
  with the RK4 stage updates fused into PSUM accumulation:
    pre_1 = W1.T x
    pre_2 = W1.T x + (h/2 * W3W1).T h2_1      (bias_2 = b1 + h/2 * W1.T b3)
    pre_3 = W1.T x + (h/2 * W3W1).T h2_2
    pre_4 = W1.T x + (h   * W3W1).T h2_3      (bias_4 = b1 + h * W1.T b3)
    S     = (h/6*W3).T h2_1 + (h/3*W3).T h2_2 + (h/3*W3).T h2_3 + (h/6*W3).T h2_4
    x'    = x + S + h*b3
All matmuls run with float16 operands (fp32 accumulate in PSUM):
validated end-to-end rel err ~6e-4 vs fp32 reference (numpy sim).
float32r was measured on HW to run at fp32 speed (~4 cyc/col,
fp32_mode=HIGH); fp16 streams 1 col/cycle and gets fast weight load.
The x state itself propagates in full fp32; a rounded fp16 copy feeds
the matmuls.

PSUM budget (8 banks, 2 chunks): per chunk 2 pre + 1 E + 1 S slots.
The pre-bank mmU (W1.T x) restarts are emitted just-in-time (one eval
ahead of their accumulate) so only 2 pre banks per chunk are ever live.
"""

import numpy as np

M = 4096
N_STEPS = 199  # N-1
H = 128
N_CORES = 8
B_CORE = M // N_CORES          # 512 trajectories per core
CHUNKS = 2
B_CHUNK = B_CORE // CHUNKS     # 256 columns per chunk

_compiled = None


def _enable_ldw_opt():
    import os
    if not os.environ.get("BASS_LDW_OPT"):
        return
    import concourse.bass_utils as bu
    if getattr(bu, "_ldw_opt_patched", False):
        return
    orig = bu.run_command
    def patched(argv, **kw):
        argv = ["--enable-ldw-opt=true" if a == "--enable-ldw-opt=false" else a
                for a in argv]
        return orig(argv, **kw)
    bu.run_command = patched
    bu._ldw_opt_patched = True


def _build_program():
    from contextlib import ExitStack

    import concourse.bacc as bacc
    import concourse.tile as tile
    from concourse import mybir

    f32 = mybir.dt.float32
    f16 = mybir.dt.float16
    Alu = mybir.AluOpType
    Act = mybir.ActivationFunctionType

    _enable_ldw_opt()
    nc = bacc.Bacc(
        "TRN2",
        target_bir_lowering=False,
        debug=False,
        enable_asserts=True,
        num_devices=N_CORES,
    )

    # ---- DRAM I/O ----
    x0T_d = nc.dram_tensor("x0T", [2, B_CORE], f32, kind="ExternalInput").ap()
    w1_d = nc.dram_tensor("w1", [2, H], f16, kind="ExternalInput").ap()
    w2_d = nc.dram_tensor("w2", [H, H], f16, kind="ExternalInput").ap()
    # per-step scaled (W3@W1): [n] -> (h/2)*Wf ; and h*Wf
    wfa_d = nc.dram_tensor("wfa", [N_STEPS, H, H], f16, kind="ExternalInput").ap()
    wfb_d = nc.dram_tensor("wfb", [N_STEPS, H, H], f16, kind="ExternalInput").ap()
    # per-step scaled W3 columns, interleaved [128, N_STEPS*4]:
    # cols 4n:4n+2 = (h/6)W3, 4n+2:4n+4 = (h/3)W3
    w3s_d = nc.dram_tensor("w3s", [H, N_STEPS * 4], f16, kind="ExternalInput").ap()
    # biases: [128, N_STEPS] columns; biasB = b1 + (h/2) W1.T b3, biasD = b1 + h W1.T b3
    biasA_d = nc.dram_tensor("biasA", [H, 1], f32, kind="ExternalInput").ap()
    biasB_d = nc.dram_tensor("biasB", [H, N_STEPS], f32, kind="ExternalInput").ap()
    biasD_d = nc.dram_tensor("biasD", [H, N_STEPS], f32, kind="ExternalInput").ap()
    b2_d = nc.dram_tensor("b2", [H, 1], f32, kind="ExternalInput").ap()
    hb3_d = nc.dram_tensor("hb3", [2, N_STEPS], f32, kind="ExternalInput").ap()
    # output: steps 1..199, feature-major [n, 2, B_CORE]
    y_d = nc.dram_tensor("y", [N_STEPS, 2, B_CORE], f32, kind="ExternalOutput").ap()

    with tile.TileContext(nc) as tc, ExitStack() as ctx:
        consts = ctx.enter_context(tc.tile_pool(name="consts", bufs=1))
        wf_pool = ctx.enter_context(tc.tile_pool(name="wf", bufs=3))
        act_pool = ctx.enter_context(tc.tile_pool(name="acts", bufs=4))
        x_pool = ctx.enter_context(tc.tile_pool(name="xs", bufs=4))
        psum = ctx.enter_context(tc.tile_pool(name="psum", bufs=1, space="PSUM"))

        # ---- load constants ----
        w1_s = consts.tile([2, H], f16)
        nc.sync.dma_start(w1_s[:], w1_d[:])
        w2_s = consts.tile([H, H], f16)
        nc.sync.dma_start(w2_s[:], w2_d[:])
        w3s_s = consts.tile([H, N_STEPS * 4], f16)
        nc.sync.dma_start(w3s_s[:], w3s_d[:])
        biasA_s = consts.tile([H, 1], f32)
        nc.sync.dma_start(biasA_s[:], biasA_d[:])
        biasB_s = consts.tile([H, N_STEPS], f32)
        nc.sync.dma_start(biasB_s[:], biasB_d[:])
        biasD_s = consts.tile([H, N_STEPS], f32)
        nc.sync.dma_start(biasD_s[:], biasD_d[:])
        b2_s = consts.tile([H, 1], f32)
        nc.sync.dma_start(b2_s[:], b2_d[:])
        hb3_s = consts.tile([2, N_STEPS], f32)
        nc.sync.dma_start(hb3_s[:], hb3_d[:])

        # initial x chunks: full-precision state + rounded fp16 copy
        xc, xrc = [], []
        for c in range(CHUNKS):
            xt = x_pool.tile([2, B_CHUNK], f32, name=f"x_c{c}", tag=f"x{c}")
            nc.sync.dma_start(xt[:], x0T_d[:, c * B_CHUNK : (c + 1) * B_CHUNK])
            xr = x_pool.tile([2, B_CHUNK], f16, name=f"xr_c{c}", tag=f"xr{c}")
            nc.vector.tensor_copy(xr[:], xt[:])
            xc.append(xt)
            xrc.append(xr)

        def mm(out, lhsT, rhs, start, stop):
            nc.tensor.matmul(out, lhsT, rhs, start=start, stop=stop)

        # per-chunk step state machines, advanced eval-by-eval interleaved
        class ChunkStep:
            def __init__(self, c, n, wfa, wfb):
                self.c, self.n = c, n
                self.wfa, self.wfb = wfa, wfb
                self.pre = [None] * 5  # pre banks 1..4
                self.S = None
                self.h2 = None

            def t(self, pool_tag, shape, dtype, nm):
                bufs = {"pre": 2, "e": 1, "s": 1}[pool_tag]
                return psum.tile(
                    shape, dtype, name=f"{nm}_{self.n}_{self.c}",
                    tag=f"{pool_tag}{self.c}", bufs=bufs,
                )

            def eval_(self, i):
                c, n = self.c, self.n
                bB = biasB_s[:, n : n + 1]
                bD = biasD_s[:, n : n + 1]
                if i == 1:
                    self.pre[1] = self.t("pre", [H, B_CHUNK], f32, "U")
                    mm(self.pre[1][:], w1_s[:], xrc[c][:], start=True, stop=False)
                    self.pre[2] = self.pre[1]
                    self.S = self.t("s", [2, B_CHUNK], f32, "S")
                elif i < 4:
                    self.pre[i + 1] = self.t("pre", [H, B_CHUNK], f32, f"P{i+1}")
                    mm(self.pre[i + 1][:], w1_s[:], xrc[c][:],
                       start=True, stop=False)
                bias = {1: biasA_s[:, 0:1], 2: bB, 3: bB, 4: bD}[i]
                h1 = act_pool.tile([H, B_CHUNK], f16, name=f"h1_{n}_{c}{i}",
                                   tag=f"h1{c}")
                if c % 2 == 0:
                    nc.scalar.activation(h1[:], self.pre[i][:], Act.Relu, bias=bias)
                else:
                    nc.vector.tensor_scalar(h1[:], self.pre[i][:], bias, 0.0,
                                            Alu.add, Alu.max)
                E = self.t("e", [H, B_CHUNK], f32, f"E{i}")
                mm(E[:], w2_s[:], h1[:], start=True, stop=True)
                h2 = act_pool.tile([H, B_CHUNK], f16, name=f"h2_{n}_{c}{i}",
                                   tag=f"h2{c}")
                if c % 2 == 0:
                    nc.vector.tensor_scalar(h2[:], E[:], b2_s[:, 0:1], 0.0,
                                            Alu.add, Alu.max)
                else:
                    nc.scalar.activation(h2[:], E[:], Act.Relu, bias=b2_s[:, 0:1])
                if i < 4:
                    wf = self.wfa if i < 3 else self.wfb
                    mm(self.pre[i + 1][:], wf[:], h2[:], start=False, stop=True)
                w3col = w3s_s[:, 4 * n : 4 * n + 2] if i in (1, 4) \
                    else w3s_s[:, 4 * n + 2 : 4 * n + 4]
                mm(self.S[:], w3col[:], h2[:], start=(i == 1), stop=(i == 4))

            def finish(self):
                c, n = self.c, self.n
                hb3c = hb3_s[:, n : n + 1]
                xnr = x_pool.tile([2, B_CHUNK], f16, name=f"xr_{n}_{c}",
                                  tag=f"xr{c}")
                nc.vector.scalar_tensor_tensor(
                    xnr[:], self.S[:], hb3c, xc[c][:], Alu.add, Alu.add
                )
                xn = x_pool.tile([2, B_CHUNK], f32, name=f"x_{n}_{c}", tag=f"x{c}")
                nc.vector.scalar_tensor_tensor(
                    xn[:], self.S[:], hb3c, xc[c][:], Alu.add, Alu.add
                )
                nc.sync.dma_start(
                    y_d[n, :, c * B_CHUNK : (c + 1) * B_CHUNK], xn[:]
                )
                xc[c] = xn
                xrc[c] = xnr

        for n in range(N_STEPS):
            wfa = wf_pool.tile([H, H], f16, name=f"wfa_{n}", tag="wfa")
            nc.sync.dma_start(wfa[:], wfa_d[n])
            wfb = wf_pool.tile([H, H], f16, name=f"wfb_{n}", tag="wfb")
            nc.sync.dma_start(wfb[:], wfb_d[n])
            steps = [ChunkStep(c, n, wfa, wfb) for c in range(CHUNKS)]
            for i in (1, 2, 3, 4):
                for s in steps:
                    s.eval_(i)
            for s in steps:
                s.finish()

    nc.compile()
    return nc


def _prep_inputs(x0, t, W1, b1, W2, b2, W3, b3):
    """Host-side derived tensors (weights fp16, biases fp32)."""
    f32 = np.float32
    f16 = np.float16
    hs = (t[1:] - t[:-1]).astype(f32)  # [199], same op order as reference
    Wf = (W3.astype(np.float64) @ W1.astype(np.float64))  # [128,128]
    wfa = np.empty((N_STEPS, H, H), f16)
    wfb = np.empty((N_STEPS, H, H), f16)
    w3s = np.empty((H, N_STEPS * 4), f16)
    biasB = np.empty((H, N_STEPS), f32)
    biasD = np.empty((H, N_STEPS), f32)
    hb3 = np.empty((2, N_STEPS), f32)
    w1b3 = (W1.astype(np.float64).T @ b3.astype(np.float64))  # [128]
    b1_64 = b1.astype(np.float64)
    W3_64 = W3.astype(np.float64)
    for n in range(N_STEPS):
        h = float(hs[n])
        wfa[n] = ((h / 2.0) * Wf).astype(f16)
        wfb[n] = (h * Wf).astype(f16)
        w3s[:, 4 * n : 4 * n + 2] = ((h / 6.0) * W3_64).astype(f16)
        w3s[:, 4 * n + 2 : 4 * n + 4] = ((h / 3.0) * W3_64).astype(f16)
        biasB[:, n] = (b1_64 + (h / 2.0) * w1b3).astype(f32)
        biasD[:, n] = (b1_64 + h * w1b3).astype(f32)
        hb3[:, n] = (h * b3.astype(np.float64)).astype(f32)
    shared = {
        "w1": np.ascontiguousarray(W1.astype(f16)),
        "w2": np.ascontiguousarray(W2.astype(f16)),
        "wfa": wfa,
        "wfb": wfb,
        "w3s": w3s,
        "biasA": np.ascontiguousarray(b1.astype(f32).reshape(H, 1)),
        "biasB": biasB,
        "biasD": biasD,
        "b2": np.ascontiguousarray(b2.astype(f32).reshape(H, 1)),
        "hb3": hb3,
    }
    in_maps = []
    for c in range(N_CORES):
        m = dict(shared)
        m["x0T"] = np.ascontiguousarray(
            x0[c * B_CORE : (c + 1) * B_CORE].astype(f32).T
        )
        in_maps.append(m)
    return in_maps


def kernel(x0, t, W1, b1, W2, b2, W3, b3):
    global _compiled
    from concourse.bass_utils import run_bass_kernel_spmd

    if _compiled is None:
        _compiled = _build_program()
    nc = _compiled

    in_maps = _prep_inputs(x0, t, W1, b1, W2, b2, W3, b3)
    res = run_bass_kernel_spmd(nc, in_maps, list(range(N_CORES))).results

    out = np.empty((N_STEPS + 1, M, 2), np.float32)
    out[0] = x0
    for c in range(N_CORES):
        y = res[c]["y"]  # [199, 2, 512]
        out[1:, c * B_CORE : (c + 1) * B_CORE, :] = y.transpose(0, 2, 1)
    return out

